# revision 26
# baseline (speedup 1.0000x reference)
"""Trainium2 Bass kernel for nn_Attention_KV (dense transformer attention
with K=Q sharing and a linear positional bias), distributed over 8 cores.

Sharding: 2 batch-groups x 4 query-quarters (collective-free). Core
c = 4*g + s owns batches 4g..4g+3 and query rows i in [256*s, 256*(s+1)).
The positional bias pos_bias(i,j) is head/batch independent but sharded
by i-quarter, so each core loads exactly the pos slice it consumes.

Layout tricks (all pure host-side layout, no host math):
  - The token (j) axis is ROLLED per core so the core's own query
    quarter occupies columns 0:IQ of k^T. The query block is then just a
    slice of kT - no separate x_q input, no separate k_q matmuls - while
    keeping the program identical across cores (SPMD). j only ever
    appears inside sums, so the roll is invisible in the output.
  - pos is pre-transposed to [jt, j, i, p] blocks so each j-tile loads
    as one fully contiguous 3.2MB HBM stream.

All attention math keeps scores TRANSPOSED (keys j on partitions,
queries i on the free axis); dots = k @ k^T is symmetric so this is
free, and softmax + the attn @ v contraction need no on-chip transpose:
  - scores^T = c*dots lands in PSUM (c = scale*sum(w_pos))
  - es = Exp(scale=c)(dots) * P where P = exp(pos_bias^T) is computed
    ONCE per core (instead of re-adding pos via identity matmuls for
    every batch*head)
  - attn@v as lhsT = v_ext (ones column appended -> row 64 of the
    result is the softmax denominator Z), rhs = es
  - 1/Z = exp(-ln Z) on the Scalar engine (both functions live in one
    ACT table set), broadcast across partitions by GpSimd, folded into
    the PSUM->SBUF copy of U

Scheduling: engines execute their instruction streams IN ORDER, so PE
gaps in the attention phase (waiting on exp/mult of the scores) are
filled STATICALLY by weaving the next-next batch's kv matmul chunks
between attention heads (kv uses a third buffer set so no WAR stall),
and by interleaving the last two batches' attention head-by-head. This
keeps the PE busy-window dense enough to hold the HAM clock gate at
full rate. The pos-bias multiply is split DVE/GpSimd to halve the
vector-engine serial load. Everything flows in bf16 (f32 PSUM
accumulation). b_pos (a scalar added to every score) is dropped:
softmax is shift invariant.
"""

import sys

sys.path.insert(0, "/opt/trn_rl_repo")

import numpy as np

import concourse.bacc as bacc
import concourse.bass as bass
import concourse.mybir as mybir
from concourse import tile
from concourse.bass_utils import run_bass_kernel_spmd

B, N, DIM, H, POS_DIM = 8, 1024, 512, 8, 50
D = DIM // H  # 64
NC = 8  # cores
BPC = 4  # batches per core
IQ = 256  # query rows per core
JT = N // 128  # 8 j-tiles
SCALE = float(DIM) ** -0.5

F32 = mybir.dt.float32
F32R = mybir.dt.float32r
BF16 = mybir.dt.bfloat16
AX = mybir.AxisListType
ALU = mybir.AluOpType
ACTF = mybir.ActivationFunctionType

POS_CHUNK = 64  # i-columns of pos per multiply op
KVSETS = 4  # all batches' kv resident: the whole kv build is PE filler
# work for the serial pos-bias pipeline that gates attention


def build_program(reps: int = 1):
    nc = bacc.Bacc("TRN2", target_bir_lowering=False, debug=False)

    # ---- DRAM parameters (per-core) ----
    xT_d = nc.declare_dram_parameter("xT", [BPC, DIM, N], BF16, isOutput=False)
    wkvT_d = nc.declare_dram_parameter("wkvT", [DIM, 2 * DIM], BF16, isOutput=False)
    wout_d = nc.declare_dram_parameter("wout", [DIM, DIM], BF16, isOutput=False)
    bout_d = nc.declare_dram_parameter("bout", [1, DIM], BF16, isOutput=False)
    wposr_d = nc.declare_dram_parameter(
        "wposr", [128, POS_CHUNK, POS_DIM], BF16, isOutput=False
    )
    posT_d = nc.declare_dram_parameter(
        "posT", [JT, 2, 128, 128, POS_DIM], BF16, isOutput=False
    )
    y_d = nc.declare_dram_parameter("y", [BPC, IQ, DIM], F32, isOutput=True)

    with tile.TileContext(nc) as tc:
        with (
            tc.tile_pool(name="persist", bufs=1) as pp,
            tc.tile_pool(name="pos_in", bufs=2) as pos_pool,
            tc.tile_pool(name="exps", bufs=6) as epool,
            tc.tile_pool(name="rzs", bufs=2) as rzpool,
            tc.tile_pool(name="outsb", bufs=2) as opool,
            tc.tile_pool(name="mm_ps", bufs=2, space="PSUM") as mmps,
            tc.tile_pool(name="dots_ps", bufs=2, space="PSUM") as dotsps,
            tc.tile_pool(name="up_ps", bufs=2, space="PSUM") as upps,
        ):
            for _rep in range(reps):
                # ---- preload small tensors + weights ----
                wposr = pp.tile([128, POS_CHUNK, POS_DIM], BF16, tag="wposr")
                nc.sync.dma_start(wposr[:], wposr_d[:])
                wkvT = [
                    pp.tile([128, 2 * DIM], BF16, name=f"wkvT{t}", tag=f"wkvT{t}")
                    for t in range(4)
                ]
                for t in range(4):
                    nc.sync.dma_start(wkvT[t][:], wkvT_d[t * 128 : (t + 1) * 128, :])
                wout = [
                    pp.tile([64, DIM], BF16, name=f"wout{h}", tag=f"wout{h}")
                    for h in range(H)
                ]
                for h in range(H):
                    nc.sync.dma_start(wout[h][:], wout_d[h * 64 : (h + 1) * 64, :])
                bout = pp.tile([1, DIM], BF16, tag="bout")
                nc.sync.dma_start(bout[:], bout_d[:])
                ones1 = pp.tile([1, 128], BF16, tag="ones1")
                nc.vector.memset(ones1[:], 1.0)

                # c = scale * sum(w_pos) on every partition (exp scale)
                c_ap = pp.tile([128, 1], F32, tag="c_ap")
                nc.vector.tensor_reduce(c_ap[:], wposr[:, 0, :], axis=AX.X, op=ALU.add)
                nc.scalar.mul(c_ap[:], c_ap[:], SCALE)

                # v_ext tiles (all kv buffer sets), ones column set once
                vext_sets = {
                    s3: [
                        pp.tile(
                            [128, H, D + 1],
                            BF16,
                            name=f"vext{t}_{s3}",
                            tag=f"vext{t}_{s3}",
                        )
                        for t in range(JT)
                    ]
                    for s3 in range(KVSETS)
                }
                for s3 in range(KVSETS):
                    for t in range(JT):
                        nc.vector.memset(vext_sets[s3][t][:, :, D : D + 1], 1.0)
                xT_sets = {
                    s3: [
                        pp.tile([128, N], BF16, name=f"xT{t}_{s3}", tag=f"xT{t}_{s3}")
                        for t in range(4)
                    ]
                    for s3 in range(KVSETS)
                }
                kT_sets = {
                    s3: [
                        pp.tile([128, N], BF16, name=f"kT{t}_{s3}", tag=f"kT{t}_{s3}")
                        for t in range(4)
                    ]
                    for s3 in range(KVSETS)
                }

                # P = exp(pos_bias^T) for this core's i-quarter, all j
                pbias = pp.tile([128, JT, IQ], BF16, tag="pbias")
                pexp = pp.tile([128, JT, IQ], BF16, tag="pexp")

                def emit_pos_quarter(qq):
                    # pos-bias pipeline, 4 half-jt-tiles per call so it can
                    # be interleaved between the kv batches. The weight
                    # multiply alternates DVE / GpSimd per half-tile (the
                    # two engines chew one jt in parallel); the X-reduce is
                    # DVE-only.
                    for k in range(4 * qq, 4 * qq + 4):
                        jt, ih = k // 2, k % 2
                        pt = pos_pool.tile(
                            [128, 128, POS_DIM], BF16, name="pchunk", tag="pchunk"
                        )
                        nc.sync.dma_start(pt[:], posT_d[jt, ih])
                        eng = nc.vector if ih == 0 else nc.gpsimd
                        for ic in range(128 // POS_CHUNK):
                            sl = slice(ic * POS_CHUNK, (ic + 1) * POS_CHUNK)
                            eng.tensor_tensor(
                                pt[:, sl, :], pt[:, sl, :], wposr[:], op=ALU.mult
                            )
                        isl = slice(ih * 128, (ih + 1) * 128)
                        with nc.allow_low_precision(
                            reason="pos bias flows in bf16 by design"
                        ):
                            nc.vector.tensor_reduce(
                                pbias[:, jt, isl], pt[:], axis=AX.X, op=ALU.add
                            )
                        if ih == 1:
                            nc.scalar.activation(
                                pexp[:, jt, :], pbias[:, jt, :], ACTF.Exp
                            )

                # ---- kv: x load + k^T / v_ext builds, split into chunks ----
                kv_tiles = {}

                def emit_kv_dma(b):
                    s3 = b % KVSETS
                    xT = xT_sets[s3]
                    for t in range(4):
                        nc.sync.dma_start(
                            xT[t][:], xT_d[b, t * 128 : (t + 1) * 128, :]
                        )
                    kv_tiles[b] = (kT_sets[s3], vext_sets[s3])

                def kv_chunks(b):
                    """8 closures, each ~8 matmuls: 4 kT column groups then
                    4 v_ext pairs. Woven between attention heads so the PE
                    always has independent ready work during softmax waits."""
                    s3 = b % KVSETS
                    xT, kT, vext = xT_sets[s3], kT_sets[s3], vext_sets[s3]

                    def kt_group(t):
                        def emit():
                            pss = [
                                mmps.tile([128, 512], F32, name="mmtile", tag="mm")
                                for _ in range(2)
                            ]
                            for dc in range(4):
                                for nchunk in range(2):
                                    nc.tensor.matmul(
                                        pss[nchunk][:],
                                        wkvT[dc][:, t * 128 : (t + 1) * 128],
                                        xT[dc][:, nchunk * 512 : (nchunk + 1) * 512],
                                        start=(dc == 0),
                                        stop=(dc == 3),
                                    )
                            for nchunk in range(2):
                                nc.vector.tensor_copy(
                                    kT[t][:, nchunk * 512 : (nchunk + 1) * 512],
                                    pss[nchunk][:],
                                )

                        return emit

                    def vext_pair(p):
                        def emit():
                            for nt in (2 * p, 2 * p + 1):
                                ps = mmps.tile(
                                    [128, 512], F32, name="mmtile", tag="mm"
                                )
                                for dc in range(4):
                                    nc.tensor.matmul(
                                        ps[:],
                                        xT[dc][:, nt * 128 : (nt + 1) * 128],
                                        wkvT[dc][:, DIM : 2 * DIM],
                                        start=(dc == 0),
                                        stop=(dc == 3),
                                    )
                                nc.scalar.copy(
                                    vext[nt][:, :, 0:D],
                                    ps[:].rearrange("p (h d) -> p h d", h=H),
                                )

                        return emit

                    return [kt_group(t) for t in range(4)] + [
                        vext_pair(p) for p in range(4)
                    ]

                def emit_kv(b):
                    emit_kv_dma(b)
                    for chunk in kv_chunks(b):
                        chunk()

                # ---- attention ----
                UT_sets = {
                    s2: [
                        pp.tile([64, IQ], BF16, name=f"UT{h}_{s2}", tag=f"UT{h}_{s2}")
                        for h in range(H)
                    ]
                    for s2 in (0, 1)
                }

                def head_scores(b, h):
                    """dots + exp + pos multiply for both jt-groups; returns
                    the two es tiles. No up-matmuls yet - the caller weaves
                    independent PE work between scores and up."""
                    kT, _ = kv_tiles[b]
                    kt = kT[h // 2]
                    pr = slice(64 * (h % 2), 64 * (h % 2) + 64)
                    ess = []
                    for jg in range(JT // 4):
                        dots = dotsps.tile(
                            [128, 4 * IQ], F32, name="dotstile", tag="dots"
                        )
                        for q in range(4):
                            jt = jg * 4 + q
                            qsl = slice(q * IQ, (q + 1) * IQ)
                            nc.tensor.matmul(
                                dots[:, qsl],
                                kt[pr, jt * 128 : (jt + 1) * 128],
                                kt[pr, 0:IQ],
                                start=True,
                                stop=True,
                            )
                        es = epool.tile([128, 4 * IQ], BF16, name="expS", tag="expS")
                        nc.scalar.activation(es[:], dots[:], ACTF.Exp, scale=c_ap[:])
                        nc.vector.tensor_tensor(
                            es[:],
                            es[:],
                            pexp[:, jg * 4 : (jg + 1) * 4, :].rearrange(
                                "p a b -> p (a b)"
                            ),
                            op=ALU.mult,
                        )
                        ess.append(es)
                    return ess

                def head_up(b, h, ess):
                    """attn @ v_ext accumulation + softmax normalization."""
                    s2 = b % 2
                    _, vext = kv_tiles[b]
                    UT = UT_sets[s2]
                    up = upps.tile([D + 1, IQ], F32, name="uptile", tag="up")
                    for jg in range(JT // 4):
                        for q in range(4):
                            jt = jg * 4 + q
                            qsl = slice(q * IQ, (q + 1) * IQ)
                            nc.tensor.matmul(
                                up[:],
                                vext[jt][:, h, :],
                                ess[jg][:, qsl],
                                start=(jt == 0),
                                stop=(jt == JT - 1),
                            )
                    # Evacuate up to SBUF at once - the PSUM slot is the
                    # scarce resource; holding it through the whole normalize
                    # chain (reciprocal+broadcast+multiply ~5us) stalls the
                    # next head's up-matmuls and re-throttles the PE clock.
                    upc = rzpool.tile([D + 1, IQ], F32, name="upc", tag="upc")
                    nc.vector.tensor_copy(upc[:], up[:])
                    # row 64 = Z; normalize from the SBUF copy. (An exp(-ln Z)
                    # ACT variant thrashes activation-table sets - Ln and Exp
                    # resolve to different table loads - so the multi-pass DVE
                    # reciprocal stays the cheapest option.) The final scale
                    # runs on GpSimd: it is latency-tolerant (only the output
                    # projection consumes UT) and keeps DVE's queue short so
                    # the next head's es-multiply is never blocked behind it.
                    rz = rzpool.tile([1, IQ], F32, name="rz", tag="rz")
                    nc.vector.reciprocal(rz[:], upc[64:65, :])
                    rzb = rzpool.tile([64, IQ], F32, name="rzb", tag="rzb")
                    nc.gpsimd.partition_broadcast(rzb[:], rz[:])
                    nc.gpsimd.tensor_tensor(
                        UT[h][:], upc[0:64, :], rzb[:], op=ALU.mult
                    )

                def emit_attn_pair(b0, b1):
                    """attention for two batches, heads interleaved - each
                    batch's scores latency is hidden by the other's matmuls."""
                    for h in range(H):
                        ess0 = head_scores(b0, h)
                        ess1 = head_scores(b1, h)
                        head_up(b0, h, ess0)
                        head_up(b1, h, ess1)

                def emit_final(b):
                    s2 = b % 2
                    UT = UT_sets[s2]
                    for it in range(IQ // 128):
                        isl = slice(it * 128, (it + 1) * 128)
                        fps = mmps.tile([128, 512], F32, name="mmtile", tag="mm")
                        for h in range(H):
                            nc.tensor.matmul(
                                fps[:],
                                UT[h][:, isl],
                                wout[h][:],
                                start=(h == 0),
                                stop=False,
                            )
                        nc.tensor.matmul(
                            fps[:], ones1[:], bout[:], start=False, stop=True
                        )
                        ot = opool.tile([128, 512], F32, name="osb", tag="osb")
                        nc.vector.tensor_copy(ot[:], fps[:])
                        nc.sync.dma_start(y_d[b, isl, :], ot[:])

                emit_kv(0)
                emit_pos_quarter(0)
                emit_kv(1)
                emit_pos_quarter(1)
                emit_kv(2)
                emit_pos_quarter(2)
                emit_kv(3)
                emit_pos_quarter(3)
                emit_attn_pair(0, 1)
                emit_final(0)
                emit_final(1)
                emit_attn_pair(2, 3)
                emit_final(2)
                emit_final(3)

    nc.compile()
    return nc


_CACHE = {}


def _get_program():
    if "nc" not in _CACHE:
        _CACHE["nc"] = build_program()
    return _CACHE["nc"]


def _host_shard(x, pos, W_kv, W_out, b_out, w_pos, b_pos):
    """Build the 8 per-core input maps (pure layout work, no math)."""
    import ml_dtypes

    bf16 = ml_dtypes.bfloat16
    x = np.asarray(x, dtype=np.float32)
    pos = np.asarray(pos, dtype=np.float32)
    W_kv = np.asarray(W_kv, dtype=np.float32)
    W_out = np.asarray(W_out, dtype=np.float32)
    b_out = np.asarray(b_out, dtype=np.float32)
    w_pos = np.asarray(w_pos, dtype=np.float32)

    wkvT = np.ascontiguousarray(W_kv.T.astype(bf16))  # (512, 1024)
    wout = np.ascontiguousarray(W_out.T.astype(bf16))  # (512, 512)
    boutr = np.ascontiguousarray(b_out.reshape(1, DIM).astype(bf16))
    wposr = np.ascontiguousarray(
        np.broadcast_to(w_pos.astype(bf16), (128, POS_CHUNK, POS_DIM))
    )

    in_maps = []
    for c in range(NC):
        g, s = c // 4, c % 4
        bs = slice(4 * g, 4 * g + BPC)
        isl = slice(s * IQ, (s + 1) * IQ)
        # roll tokens so this core's query quarter is columns 0:IQ
        xr = np.roll(x[bs], -s * IQ, axis=1)  # (4, 1024, 512)
        xT = np.ascontiguousarray(xr.transpose(0, 2, 1).astype(bf16))  # (4,512,1024)
        posr = np.roll(pos[0, isl, :, :], -s * IQ, axis=1)  # (256 i, 1024 j, 50)
        posT = np.ascontiguousarray(
            posr.transpose(1, 0, 2)
            .reshape(JT, 128, 2, 128, POS_DIM)
            .transpose(0, 2, 1, 3, 4)  # (jt, i-half, j, i, p) blocks
            .astype(bf16)
        )
        in_maps.append(
            {
                "xT": xT,
                "wkvT": wkvT,
                "wout": wout,
                "bout": boutr,
                "wposr": wposr,
                "posT": posT,
            }
        )
    return in_maps


def kernel(**inputs) -> np.ndarray:
    nc = _get_program()
    in_maps = _host_shard(**inputs)
    res = run_bass_kernel_spmd(nc, in_maps, list(range(NC)))
    out = np.empty((B, N, DIM), dtype=np.float32)
    for c in range(NC):
        g, s = c // 4, c % 4
        out[4 * g : 4 * g + BPC, s * IQ : (s + 1) * IQ, :] = res.results[c]["y"]
    return out


if __name__ == "__main__":
    import reference

    inputs = {k: np.asarray(v) for k, v in reference.setup_inputs().items()}
    expected = np.asarray(reference.reference(**inputs))
    actual = kernel(**inputs)
    err = np.abs(actual - expected).max()
    rel = err / np.abs(expected).max()
    print(f"absmax err: {err:.3e}  rel: {rel:.3e}")
